# revision 4
# baseline (speedup 1.0000x reference)
"""Channel-attention (XCA-style) kernel for TRN2, 8 cores, 1 image/core.

v2: fp8 DoubleRow tensor-engine pipeline.
  q  = conv3x3(y, Wq')   fp8 DR, weights stationary, out [c, n] -> qT fp8
  kv1 = 1x1(x, Wkv)      fp8 DR with hi/lo residual (3 terms, exact-ish)
  k  = dw3x3(kv1_k)      fp8 DR block-diag matmul -> kT [n, c] directly
  v  = dw3x3(kv1_v)      fp16 on DVE/ACT (accuracy: v path cannot take fp8)
  G|QQ|KK = DR gram on qT/kT fp8; norms = diag(QQ), diag(KK)
  A  = softmax(G * t / (|q||k|))  block-diag per head
  out = (P @ A) @ v      fp16, proj pre-divided by the kv scale
"""
import numpy as np

import concourse.bass as bass
import concourse.bacc as bacc
import concourse.mybir as mybir
import concourse.tile as tile
from concourse.masks import make_identity

F32 = mybir.dt.float32
FP16 = mybir.dt.float16
FP8 = mybir.dt.float8e4
DR = mybir.MatmulPerfMode.DoubleRow

B, C, H, W = 8, 192, 128, 128
HEADS = 8
CH = C // HEADS            # 24
N = H * W                  # 16384
WP2 = W + 2                # padded row stride (130)
NG = 8                     # row groups
GR = H // NG               # rows per group (16)
RT = 4                     # rows per conv tile
NT = H // RT               # conv tiles (32)
YL = (H + 2) * WP2         # padded y flat length 16900
YW = GR * WP2 + W + 2      # y group window 2210
YWP = 2224                 # y tile subtile pitch (mult of 16 for DoubleRow)
KP = 144                   # k-tile row pitch (mult of 16 for DoubleRow)

CC = [(0, 96), (96, 96)]   # head-aligned chunks for attention blocks
VB = [(0, 128), (128, 64)] # v-channel blocks

SA = 64.0   # wq fp8 pre-scale
SB = 32.0   # wkv pre-scale
SC = 32.0   # dw-k pre-scale

# tap order t = dy*3+dx, dy,dx in {0,1,2}; k-tile offset per tap
KTAP_OFF = [dy * KP + dx for dy in range(3) for dx in range(3)]
# dw-k DoubleRow pairs: dy0/dy1 same dx (step KP), dy2 alone (zero weights,
# junk partner read at -16 to satisfy the step%16==0 ISA rule)
DW_PAIRS = [(0, 3), (1, 4), (2, 5), (6, None), (7, None), (8, None)]


def build(repeat=1, parts=("q", "kv", "dwk", "dwv", "gram", "attn", "fin")):
    nc = bacc.Bacc()
    tok_in = nc.dram_tensor("tok_in", [128, 16], F32, kind="ExternalInput")
    d_y8 = nc.dram_tensor("y8p", [128, 6, YL], FP8, kind="ExternalInput")
    d_wq = nc.dram_tensor("wq8", [128, 18, C], FP8, kind="ExternalInput")
    d_x8 = nc.dram_tensor("x8p", [128, 4, N], FP8, kind="ExternalInput")
    d_wkv = nc.dram_tensor("wkv8", [128, 4, 2 * C], FP8, kind="ExternalInput")
    d_dwka = nc.dram_tensor("wdwk8a", [128, 12, 128], FP8, kind="ExternalInput")
    d_dwkb = nc.dram_tensor("wdwk8b", [64, 12, 64], FP8, kind="ExternalInput")
    d_dwv = nc.dram_tensor("wdwv", [C, 16], F32, kind="ExternalInput")
    d_wp = nc.dram_tensor("wproj", [C, C], FP16, kind="ExternalInput")
    d_tv = nc.dram_tensor("tvec", [C, 16], F32, kind="ExternalInput")
    d_msk = nc.dram_tensor("smask", [C, C], F32, kind="ExternalInput")
    d_out = nc.dram_tensor("out", [C, N], F32, kind="ExternalOutput")
    d_tok = nc.dram_tensor("tok_out", [128, 16], F32, kind="ExternalOutput")
    d_vsa = nc.dram_tensor("vspa", [128, N], FP16, kind="Internal")
    d_vsb = nc.dram_tensor("vspb", [64, N], FP16, kind="Internal")

    with tile.TileContext(nc) as tc:
        with (
            tc.tile_pool(name="wp", bufs=1) as wp,
            tc.tile_pool(name="io", bufs=2) as io,
            tc.tile_pool(name="qt", bufs=1) as qtp,
            tc.tile_pool(name="dw", bufs=2) as dwp,
            tc.tile_pool(name="sm", bufs=1) as sm,
            tc.tile_pool(name="ps", bufs=1, space="PSUM") as ps,
        ):
            tki = sm.tile([128, 16], F32)
            nc.sync.dma_start(tki, tok_in[:, :])

            t_wq = wp.tile([128, 18, C], FP8, name="wq8")
            nc.sync.dma_start(t_wq, d_wq[:, :, :])
            t_wkv = wp.tile([128, 4, 2 * C], FP8, name="wkv8")
            nc.sync.dma_start(t_wkv, d_wkv[:, :, :])
            t_dwka = wp.tile([128, 12, 128], FP8, name="dwka")
            nc.sync.dma_start(t_dwka, d_dwka[:, :, :])
            t_dwkb = wp.tile([64, 12, 64], FP8, name="dwkb")
            nc.sync.dma_start(t_dwkb, d_dwkb[:, :, :])
            t_dwv = {}
            t_dwv[0] = wp.tile([128, 16], F32, name="dwv0")
            nc.sync.dma_start(t_dwv[0], d_dwv[0:128, :])
            t_dwv[128] = wp.tile([64, 16], F32, name="dwv1")
            nc.sync.dma_start(t_dwv[128], d_dwv[128:192, :])
            t_wp_ = {}
            t_tv = {}
            t_msk = {}
            for c0, cn in CC:
                t_wp_[c0] = wp.tile([cn, C], FP16, name=f"wp{c0}")
                nc.sync.dma_start(t_wp_[c0], d_wp[c0:c0 + cn, :])
                t_tv[c0] = wp.tile([cn, 16], F32, name=f"tv{c0}")
                nc.sync.dma_start(t_tv[c0], d_tv[c0:c0 + cn, :])
                t_msk[c0] = wp.tile([cn, C], F32, name=f"msk{c0}")
                nc.sync.dma_start(t_msk[c0], d_msk[c0:c0 + cn, :])
            id16 = wp.tile([128, 128], FP16, name="id16")
            make_identity(nc, id16)
            id32 = wp.tile([128, 128], F32, name="id32")
            make_identity(nc, id32)
            ones1 = wp.tile([1, C], F32, name="ones1")
            nc.vector.memset(ones1, 1.0)

            state = {}

            def body(it=None):
                t_qT = qtp.tile([128, H, C], FP8, name="qT", tag="qT")
                t_kT = qtp.tile([128, H, C], FP8, name="kT", tag="kT")
                d_vs = {0: d_vsa, 128: d_vsb}
                # G chunks side by side: cc0 at cols 0:192, cc1 at 192:384
                pGall = ps.tile([96, 384], F32, name="pGall", tag="pGall",
                                bufs=1)
                pG = {0: pGall[:, 0:C], 96: pGall[:, C:2 * C]}
                # norms: QQ0 | QQ1 | KK0 | KK1 (96 cols each)
                pNrm = ps.tile([96, 384], F32, name="pNrm", tag="pNrm",
                               bufs=1)
                kv1 = {}

                def new_kv1(m):
                    # all tiles: padded WP2 layout (k fp8, v fp16); shifted
                    # tap reads then pick up zero borders, no edge memsets
                    ka = dwp.tile([128, 18 * KP], FP8, name="ka", tag="ka",
                                  bufs=2)
                    kb = dwp.tile([64, 18 * KP], FP8, name="kb", tag="kb",
                                  bufs=2)
                    # v tiles: +2 head/tail pad so contiguous tap reads
                    # never leave the tile
                    va = dwp.tile([128, 18 * WP2 + 4], FP16, name="va",
                                  tag="va", bufs=2)
                    vb = dwp.tile([64, 18 * WP2 + 4], FP16, name="vb",
                                  tag="vb", bufs=2)
                    kv1[m] = (ka, kb, va, vb)
                    for t in (va, vb):
                        nc.vector.memset(t[:, 0:2], 0.0)
                        nc.vector.memset(
                            t[:, 18 * WP2 + 2:18 * WP2 + 4], 0.0)
                    for ti, t in enumerate((ka, kb, va, vb)):
                        hp = 0 if ti < 2 else 2
                        pit = KP if ti < 2 else WP2
                        # zero left col + right pad (cols W+1..pitch-1)
                        eap = bass.AP(tensor=t.tensor, offset=t.offset + hp,
                                      ap=[t.ap[0], [pit, 18], [1, 1]])
                        nc.gpsimd.memset(eap, 0.0)
                        eap = bass.AP(tensor=t.tensor,
                                      offset=t.offset + hp + W + 1,
                                      ap=[t.ap[0], [pit, 18],
                                          [1, pit - W - 1]])
                        nc.gpsimd.memset(eap, 0.0)
                        if m == 0:
                            nc.gpsimd.memset(t[:, hp:hp + pit], 0.0)
                        if m == NG - 1:
                            nc.gpsimd.memset(
                                t[:, hp + 17 * pit:hp + 18 * pit], 0.0)
                    return kv1[m]

                def kv_store(pkv, ct, m, lo, nrows):
                    """Store pkv ([*, nrows*W]) as kv rows starting at local
                    row lo of group m tiles. ct = chunk (ka, kb, va, vb)."""
                    tiles = kv1.get(m) or new_kv1(m)
                    dst_t = tiles[ct]
                    hp = 0 if ct < 2 else 2
                    pit = KP if ct < 2 else WP2
                    dst = bass.AP(tensor=dst_t.tensor,
                                  offset=dst_t.offset + hp + lo * pit + 1,
                                  ap=[dst_t.ap[0], [pit, nrows], [1, W]])
                    if ct < 2:       # k chunks fp8 on ACT
                        nc.scalar.copy(dst, pkv)
                    else:            # v chunks fp16 on ACT
                        nc.scalar.copy(dst, pkv)

                def qload_group(g):
                    ty = io.tile([128, 6, YWP], FP8, name="y8g", tag="y8g",
                                 bufs=2)
                    nc.sync.dma_start(
                        ty[:, :, 0:YW],
                        d_y8[:, :, GR * g * WP2:GR * g * WP2 + YW])
                    state["y"] = ty

                def qconv_pair(g, lr):
                    """q rows 16g+lr, +1 directly in qT layout: y stationary
                    (lhsT), weights moving. One psum tile per row pair."""
                    ty = state["y"]
                    pqT = ps.tile([128, 2, C], F32, name="pqT", tag="pq0",
                                  bufs=2)
                    for half in range(2):
                        nmm = 0
                        for dx in range(3):
                            for j in range(2):
                                lhsT = bass.AP(
                                    tensor=ty.tensor,
                                    offset=ty.offset + 2 * j * YWP
                                    + (lr + half) * WP2 + dx,
                                    ap=[ty.ap[0], [YWP, 2], [1, W]])
                                nc.tensor.matmul(
                                    pqT[:, half, :], lhsT,
                                    t_wq[:, 6 * dx + 2 * j:6 * dx + 2 * j + 2,
                                         :],
                                    start=(nmm == 0), stop=False,
                                    perf_mode=DR)
                                nmm += 1
                            # subtile 4 is half-real/half-zero, subtile 5 all
                            # zero: plain matmul halves the weight load
                            lhsT = bass.AP(
                                tensor=ty.tensor,
                                offset=ty.offset + 4 * YWP
                                + (lr + half) * WP2 + dx,
                                ap=[ty.ap[0], [1, W]])
                            nc.tensor.matmul(
                                pqT[:, half, :], lhsT,
                                t_wq[:, 6 * dx + 4:6 * dx + 5, :],
                                start=False, stop=(nmm == 8))
                            nmm += 1
                    r = GR * g + lr
                    nc.scalar.copy(t_qT[:, r:r + 2, :], pqT)

                def kvconv_tile(t):
                    r0 = RT * t
                    m = t // 4
                    tx = {}
                    for hl in range(2):
                        tx[hl] = io.tile([128, 2, 512], FP8, name=f"x{hl}",
                                         tag=f"x{hl}", bufs=2)
                        nc.sync.dma_start(
                            tx[hl],
                            d_x8[:, 2 * hl:2 * hl + 2, 512 * t:512 * (t + 1)])
                    # chunks: k 0-128, k 128-64, v 0-128, v 128-64
                    for ct, (o0, on) in enumerate(
                            ((0, 128), (128, 64), (192, 128), (320, 64))):
                        pkv = ps.tile([on, 512], F32, name="pkv", tag="pkv",
                                      bufs=2)
                        terms = [(0, 0), (0, 1), (1, 0)]  # (w hi/lo, x hi/lo)
                        for i, (wi, xi) in enumerate(terms):
                            nc.tensor.matmul(
                                pkv,
                                t_wkv[:, 2 * wi:2 * wi + 2, o0:o0 + on],
                                tx[xi][:, :, :],
                                start=(i == 0), stop=(i == 2), perf_mode=DR)
                        lo = r0 - GR * m + 1
                        kv_store(pkv[:, :], ct, m, lo, RT)
                        if r0 % GR == 0 and m >= 1:
                            # first row also closes prev group's halo row 17
                            kv_store(pkv[:, 0:W], ct, m - 1, 17, 1)
                        if (r0 + RT) % GR == 0 and m + 1 <= NG - 1:
                            kv_store(pkv[:, 3 * W:4 * W], ct, m + 1, 0, 1)

                def dwk_pair(g, lr):
                    """dw conv for k -> kT rows 16g+lr, +1 via block-diag DR.
                    Row pair shares one psum tile (cols 0:192 / 192:384)."""
                    ka, kb, va, vb = kv1[g]
                    pkT = ps.tile([128, 2, C], F32, name="pkT", tag="ptp",
                                  bufs=2)
                    for half in range(2):
                        base = (lr + half) * KP
                        for src, wdw, o0, on in ((ka, t_dwka, 0, 128),
                                                 (kb, t_dwkb, 128, 64)):
                            nmm = 0
                            for ta, tb in DW_PAIRS:
                                off = KTAP_OFF[ta]
                                if tb is not None:
                                    dlt = KTAP_OFF[tb] - KTAP_OFF[ta]
                                    lhsT = bass.AP(
                                        tensor=src.tensor,
                                        offset=src.offset + base + off,
                                        ap=[src.ap[0], [dlt, 2], [1, W]])
                                    nc.tensor.matmul(
                                        pkT[:, half, o0:o0 + on], lhsT,
                                        wdw[:, 2 * nmm:2 * nmm + 2, :],
                                        start=(nmm == 0), stop=False,
                                        perf_mode=DR)
                                else:
                                    # unpaired tap: plain matmul, half the
                                    # weight load and FWL-eligible
                                    lhsT = bass.AP(
                                        tensor=src.tensor,
                                        offset=src.offset + base + off,
                                        ap=[src.ap[0], [1, W]])
                                    nc.tensor.matmul(
                                        pkT[:, half, o0:o0 + on], lhsT,
                                        wdw[:, 2 * nmm, :],
                                        start=False, stop=(nmm == 5))
                                nmm += 1
                    r = GR * g + lr
                    nc.scalar.copy(t_kT[:, r:r + 2, :], pkT)

                def dwv_group(g):
                    # contiguous ops over the padded layout (pad columns
                    # compute junk-free zeros, skipped by the spill DMA)
                    VL = GR * WP2  # 2080 elements per op
                    ka, kb, va, vb = kv1[g]
                    for (v0, vn), src in (((0, 128), va), ((128, 64), vb)):
                        wcol = t_dwv[v0]
                        vacc = dwp.tile([vn, VL], FP16, name="vacc",
                                        tag=f"vacc{v0}", bufs=2)
                        acc = vacc[:, :]

                        def tap_ap(tap):
                            dy, dx = tap // 3, tap % 3
                            o = 2 + dy * WP2 + dx - 1
                            return src[0:vn, o:o + VL]

                        nc.vector.tensor_scalar_mul(acc, tap_ap(4),
                                                    wcol[:, 4:5])
                        nt = 0
                        for tap in range(9):
                            if tap == 4:
                                continue
                            z = dwp.tile([vn, VL], FP16, name="dwz",
                                         tag="dwz", bufs=3)
                            nc.vector.tensor_scalar_mul(
                                z, tap_ap(tap), wcol[:, tap:tap + 1])
                            nt += 1
                            nc.vector.tensor_add(acc, acc, z)
                        vsrc = bass.AP(tensor=vacc.tensor,
                                       offset=vacc.offset + 1,
                                       ap=[vacc.ap[0], [WP2, GR], [1, W]])
                        nc.sync.dma_start(
                            d_vs[v0][0:vn, 2048 * g:2048 * (g + 1)], vsrc)

                def gram_pair(r):
                    """G, QQ, KK DR matmuls for qT/kT row pair (2r, 2r+1)."""
                    first = (r == 0)
                    last = (r == H // 2 - 1)
                    for i, (c0, cn) in enumerate(CC):
                        qsl = t_qT[:, 2 * r:2 * r + 2, c0:c0 + cn]
                        ksl = t_kT[:, 2 * r:2 * r + 2, c0:c0 + cn]
                        nc.tensor.matmul(pG[c0], qsl,
                                         t_kT[:, 2 * r:2 * r + 2, :],
                                         start=first, stop=last, perf_mode=DR)
                        nc.tensor.matmul(pNrm[:, 96 * i:96 * i + 96], qsl,
                                         qsl,
                                         start=first, stop=last, perf_mode=DR)
                        nc.tensor.matmul(pNrm[:, 192 + 96 * i:288 + 96 * i],
                                         ksl, ksl,
                                         start=first, stop=last, perf_mode=DR)

                # ---------------- main loop ----------------
                for g in range(NG):
                    if "q" in parts:
                        qload_group(g)
                    for t in range(4 * g, 4 * g + 4):
                        if "q" in parts:
                            qconv_pair(g, 4 * (t - 4 * g))
                            qconv_pair(g, 4 * (t - 4 * g) + 2)
                        if "kv" in parts:
                            kvconv_tile(t)
                    if g >= 1:
                        if "dwk" in parts:
                            for lr in range(0, GR, 2):
                                dwk_pair(g - 1, lr)
                                if "gram" in parts:
                                    gram_pair((GR * (g - 1) + lr) // 2)
                        if "dwv" in parts:
                            dwv_group(g - 1)
                if "dwk" in parts:
                    for lr in range(0, GR, 2):
                        dwk_pair(NG - 1, lr)
                        if "gram" in parts:
                            gram_pair((GR * (NG - 1) + lr) // 2)
                if "dwv" in parts:
                    dwv_group(NG - 1)
                if "attn" not in parts:
                    return

                # ---------------- attention ----------------
                rq = {}
                kn2 = {}
                for i, (c0, cn) in enumerate(CC):
                    scr = sm.tile([cn, 96], FP16, name=f"scr{c0}",
                                  tag=f"scr{c0}")
                    s = sm.tile([cn, 1], F32, name=f"qn2_{c0}",
                                tag=f"qn2_{c0}")
                    nc.vector.scalar_tensor_tensor(
                        scr, pNrm[:, 96 * i:96 * i + 96], 1.0,
                        id32[0:cn, 0:96],
                        mybir.AluOpType.mult, mybir.AluOpType.mult,
                        accum_out=s)
                    nc.scalar.sqrt(s, s)
                    nc.vector.reciprocal(s, s)
                    nc.vector.tensor_mul(s, s, t_tv[c0][:, 0:1])
                    rq[c0] = s
                    scr2 = sm.tile([cn, 96], FP16, name=f"scr2{c0}",
                                   tag=f"scr{c0}")
                    s2 = sm.tile([cn, 1], F32, name=f"kn2_{c0}",
                                 tag=f"kn2_{c0}")
                    nc.vector.scalar_tensor_tensor(
                        scr2, pNrm[:, 192 + 96 * i:288 + 96 * i], 1.0,
                        id32[0:cn, 0:96],
                        mybir.AluOpType.mult, mybir.AluOpType.mult,
                        accum_out=s2)
                    kn2[c0] = s2
                pkrow = ps.tile([1, C], F32, name="pkrow", tag="ptp", bufs=2)
                nc.tensor.transpose(pkrow[0:1, 0:96], kn2[0],
                                    id32[0:96, 0:96])
                nc.tensor.transpose(pkrow[0:1, 96:192], kn2[96],
                                    id32[0:96, 0:96])
                krow = sm.tile([1, C], F32, name="krow", tag="krow")
                nc.scalar.copy(krow, pkrow)
                nc.scalar.sqrt(krow, krow)
                nc.vector.reciprocal(krow, krow)
                rkb = {}
                for c0, cn in CC:
                    pb = ps.tile([cn, C], F32, name=f"prkb{c0}", tag="pkv",
                                 bufs=2)
                    nc.tensor.matmul(pb, ones1[0:1, c0:c0 + cn], krow,
                                     start=True, stop=True)
                    sb_ = sm.tile([cn, C], F32, name=f"rkb{c0}",
                                  tag=f"rkb{c0}")
                    nc.scalar.copy(sb_, pb)
                    rkb[c0] = sb_
                A = {}
                for c0, cn in CC:
                    s = sm.tile([cn, C], F32, name=f"S{c0}", tag=f"S{c0}")
                    nc.vector.tensor_scalar_mul(s, pG[c0][:, 0:C], rq[c0])
                    nc.vector.tensor_mul(s, s, rkb[c0])
                    nc.vector.tensor_add(s, s, t_msk[c0])
                    m = sm.tile([cn, 1], F32, name=f"m{c0}", tag=f"m{c0}")
                    a = sm.tile([cn, C], FP16, name=f"A{c0}", tag=f"A{c0}")
                    z = sm.tile([cn, 1], F32, name=f"z{c0}", tag=f"z{c0}")
                    nc.vector.tensor_reduce(
                        m, s, axis=mybir.AxisListType.X,
                        op=mybir.AluOpType.max)
                    nc.vector.tensor_scalar_mul(m, m, -1.0)
                    nc.scalar.activation(
                        a, s, mybir.ActivationFunctionType.Exp,
                        bias=m, scale=1.0, accum_out=z)
                    nc.vector.reciprocal(z, z)
                    nc.vector.tensor_scalar_mul(a, a, z)
                    A[c0] = a
                CT = {}
                for d0, dn in VB:
                    pc = ps.tile([dn, C], F32, name=f"pCT{d0}", tag="pkv",
                                 bufs=2)
                    for i, (c0, cn) in enumerate(CC):
                        nc.tensor.matmul(
                            pc, A[c0][:, d0:d0 + dn], t_wp_[c0],
                            start=(i == 0), stop=(i == 1))
                    ct_ = sm.tile([dn, C], FP16, name=f"CT{d0}",
                                  tag=f"CT{d0}")
                    nc.scalar.copy(ct_, pc)
                    CT[d0] = ct_

                # ---------------- final conv ----------------
                if "fin" not in parts:
                    return
                for t in range(NT):
                    vin = {}
                    for d0, dn in VB:
                        vin[d0] = io.tile([dn, 512], FP16, name=f"vin{d0}",
                                          tag=f"vin{d0}", bufs=2)
                        nc.scalar.dma_start(
                            vin[d0], d_vs[d0][0:dn, 512 * t:512 * (t + 1)])
                    for o0, on in CC:
                        pf = ps.tile([on, 512], F32, name=f"pf{o0}",
                                     tag="pq0", bufs=2)
                        for i, (d0, dn) in enumerate(VB):
                            nc.tensor.matmul(
                                pf, CT[d0][:, o0:o0 + on], vin[d0],
                                start=(i == 0), stop=(i == 1))
                        ost = io.tile([on, 512], F32, name=f"ost{o0}",
                                      tag=f"ost{o0}", bufs=2)
                        nc.scalar.copy(ost, pf)
                        nc.scalar.dma_start(
                            d_out[o0:o0 + on, 512 * t:512 * (t + 1)], ost)

            for it in range(repeat):
                body(it)

            o16 = sm.tile([128, 16], F32, name="o16", tag="o16")
            nc.vector.tensor_copy(o16, tki)
            nc.sync.dma_start(d_tok[:, :], o16)

    nc.compile()
    return nc


# ---------------------------------------------------------------------------
# host-side packing
# ---------------------------------------------------------------------------

def _f8(a):
    import ml_dtypes
    return np.asarray(a, np.float32).astype(ml_dtypes.float8_e4m3)


def prep_weights(kv_w, kv_dw_w, q_w, q_dw_w, proj_w, temperature):
    kv_w = np.asarray(kv_w, np.float32).reshape(2 * C, C)
    kv_dw_w = np.asarray(kv_dw_w, np.float32).reshape(2 * C, 9)
    q_w = np.asarray(q_w, np.float32).reshape(C, C)
    q_dw_w = np.asarray(q_dw_w, np.float32).reshape(C, C, 9)
    proj_w = np.asarray(proj_w, np.float32).reshape(C, C)
    temperature = np.asarray(temperature, np.float32).reshape(HEADS)

    wq = np.einsum('oct,ci->oit', q_dw_w, q_w) * SA      # [o, i, tap]
    # wq8 [128, 18, C]: subtile s18 = dx*6 + ss; K-row r = 128*ss + p
    wq8 = np.zeros((128, 18, C), np.float32)
    for dx in range(3):
        for ss in range(6):
            for p in range(128):
                r = 128 * ss + p
                if r >= 3 * C:
                    continue
                dy, ci = r // C, r % C
                wq8[p, 6 * dx + ss, :] = wq[:, ci, dy * 3 + dx]
    wq8 = _f8(wq8)

    wkv_s = kv_w * SB
    wkv_hi = _f8(wkv_s)
    wkv_lo = _f8(wkv_s - wkv_hi.astype(np.float32))
    wkv8 = np.zeros((128, 4, 2 * C), np.float32)
    for s in range(2):
        nrow = 128 if s == 0 else C - 128
        wkv8[:nrow, 0 + s, :] = \
            wkv_hi.astype(np.float32)[:, 128 * s:128 * s + nrow].T
        wkv8[:nrow, 2 + s, :] = \
            wkv_lo.astype(np.float32)[:, 128 * s:128 * s + nrow].T
    wkv8 = _f8(wkv8)

    wdwk = kv_dw_w[:C] * SC                              # [C, 9]
    wdwk8a = np.zeros((128, 12, 128), np.float32)
    wdwk8b = np.zeros((64, 12, 64), np.float32)
    for i, (ta, tb) in enumerate(DW_PAIRS):
        wdwk8a[:, 2 * i, :] = np.diag(wdwk[0:128, ta])
        wdwk8b[:, 2 * i, :] = np.diag(wdwk[128:192, ta])
        if tb is not None:
            wdwk8a[:, 2 * i + 1, :] = np.diag(wdwk[0:128, tb])
            wdwk8b[:, 2 * i + 1, :] = np.diag(wdwk[128:192, tb])
    wdwk8a = _f8(wdwk8a)
    wdwk8b = _f8(wdwk8b)

    wdwv = np.zeros((C, 16), np.float32)
    wdwv[:, :9] = kv_dw_w[C:, :9]

    wproj_T = np.ascontiguousarray(proj_w.T) / SB        # [c, o] / SB
    tvec = np.zeros((C, 16), np.float32)
    tvec[:, 0] = np.repeat(temperature, CH)
    smask = np.full((C, C), -60000.0, np.float32)
    for h in range(HEADS):
        smask[h * CH:(h + 1) * CH, h * CH:(h + 1) * CH] = 0.0
    return (wq8, wkv8, wdwk8a, wdwk8b, wdwv,
            wproj_T.astype(np.float16), tvec, smask)


def prep_image(xi, yi):
    xi = np.asarray(xi, np.float32).reshape(C, N)
    x_hi = _f8(xi)
    x_lo = _f8(xi - x_hi.astype(np.float32))
    x8p = np.zeros((128, 4, N), np.float32)
    for hl, src in enumerate((x_hi, x_lo)):
        s32 = src.astype(np.float32)
        x8p[:, 2 * hl + 0, :] = s32[0:128]
        x8p[0:64, 2 * hl + 1, :] = s32[128:192]
    x8p = _f8(x8p)

    yp = np.zeros((C, H + 2, WP2), np.float32)
    yp[:, 1:1 + H, 1:1 + W] = np.asarray(yi, np.float32).reshape(C, H, W)
    ypf = _f8(yp).astype(np.float32).reshape(C, YL)
    y8p = np.zeros((128, 6, YL), np.float32)
    for s in range(6):
        for p in range(128):
            r = 128 * s + p
            if r >= 3 * C:
                continue
            dy, ci = r // C, r % C
            ln = YL - dy * WP2
            y8p[p, s, :ln] = ypf[ci, dy * WP2:]
    return x8p, _f8(y8p)


_CACHE = {}


def make_in_maps(x, y, kv_w, kv_dw_w, q_w, q_dw_w, proj_w, temperature):
    x = np.asarray(x, np.float32)
    y = np.asarray(y, np.float32)
    (wq8, wkv8, wdwk8a, wdwk8b, wdwv, wpj, tv, smask) = prep_weights(
        kv_w, kv_dw_w, q_w, q_dw_w, proj_w, temperature)
    tok = np.zeros((128, 16), np.float32)
    in_maps = []
    for b in range(B):
        x8p, y8p = prep_image(x[b], y[b])
        in_maps.append({
            "tok_in": tok, "y8p": y8p, "x8p": x8p,
            "wq8": wq8, "wkv8": wkv8, "wdwk8a": wdwk8a, "wdwk8b": wdwk8b,
            "wdwv": wdwv, "wproj": wpj, "tvec": tv, "smask": smask,
        })
    return in_maps


def kernel(x, y, kv_w, kv_dw_w, q_w, q_dw_w, proj_w, temperature):
    in_maps = make_in_maps(x, y, kv_w, kv_dw_w, q_w, q_dw_w, proj_w,
                           temperature)
    if "nc" not in _CACHE:
        _CACHE["nc"] = build()
    nc = _CACHE["nc"]
    from concourse.bass_utils import run_bass_kernel_spmd
    res = run_bass_kernel_spmd(nc, in_maps, core_ids=list(range(B)))
    out = np.stack([res.results[b]["out"].reshape(C, H, W) for b in range(B)])
    return out.astype(np.float32)
